# revision 34
# baseline (speedup 1.0000x reference)
"""BitLinear (ternary-weight + 8-bit-activation quantized matmul) on 8 TRN2 cores.

Strategy: data-parallel over tokens. Each core gets 2048 of the 16384 tokens
plus the full weight matrix, computes the whole BitLinear forward for its
token shard on device, and the host concatenates the shards.

Math (must match the jax reference):
  w_scale = max(mean(|W|), 1e-6)                       (scalar)
  w_q     = clip(round(W / w_scale), -1, 1)            (ternary)
  a       = clip(max_i |x|, 1e-8, inf)                 (per token)
  x_q     = clip(round(x * 127 / a), -127, 127)        (8-bit ints)
  y       = (x_q @ w_q^T) * w_scale * a / 127

Final schedule (~318-322us measured vs 394us v1 baseline; roofline: 221us
bf16 GEMM + ~49us W stream + ~48us quantize window + fixed ~18us of
framework preamble/teardown -> ~306us floor):
  - w_scale is extremely sensitive (2e-4 rel deviation flips ternary weights
    near .5 boundaries -> 3e-2 err), so pass 1 must abs-sum the full fp32 W.
    W is FULLY RESIDENT in fp32 (128KB of ~207KB/partition SBUF): zero
    re-read; the host pre-tiles it to [128, 16*2048] so it streams once as
    8 x 2MB DMAs. Few DMAs matter: Tile has ~9 DMA completion-sem lanes and
    a 10th+ in-flight dma_start stalls its *issue* on an earlier DMA's full
    completion (16x1MB pushed the last W arrival from ~54us to ~71us).
  - ALL bulk transfers ride the sync/HWDGE ring and its FIFO order IS the
    schedule: x0, x1 first (prep runs during pass-1), then W, then x2, x3
    behind it. Steady-state x loads ride the scalar HWDGE ring so a prep
    DMA-transpose is never serialized behind an in-flight 1MB x load.
    SWDGE/gpsimd DMAs are useless here: compute ops on gpsimd take ~29us
    per [128,2048] tile and cast-DMAs only get ~70-100 GB/s beside the W
    stream, arriving late and (via the static scheduler's optimistic DMA
    model) head-of-line blocking the DVE queue.
  - pass-1 abs-sums alternate DVE reduce_sum (even j, non-clobbering) and
    ACT Abs->bf16 scratch in the idle ys ring with accum_out (odd j;
    bf16-rounded |W| sums are random-error ~1e-6 rel over 4.2M elements)
    inside tc.high_priority() so stray x-prep ops cannot park ahead of
    them on an engine; w_scale lands ~3us after the last W chunk.
  - rounding is the fp32 magic-number trick (+1.5*2^23, exact RNE; a bf16
    magic of +192 double-rounds and flips ~16 weights at the .5 boundary =
    up to ~1.8e-2 err, rejected). Quantize runs on [128,1024] halves
    through a shared 2-buf t1 pool. W: ACT magic -> DVE subtract into fp8
    (ints <= 8 exact in e4m3) -> full-row in-place fp8 clamp on DVE. x:
    ACT magic -> DVE subtract to bf16, no clamp needed (|x*127/a| <= 127
    by construction). x stays f32 end-to-end (3.0e-3 total err).
  - the quantize window (~48us) is BOTH-engine-saturated (ACT ~2.5us/b,
    DVE ~2.6us/b incl clamp, + x2 prep + w_scale chain) - measured at its
    floor; 3 elementwise passes over W are irreducible with 2 engines
    (ACT cannot clamp: no min/clip activation function).
  - GEMM ramp: 8 PSUM cells (tiles 0-1 x 4 col-blocks) accumulate each b
    the moment its wq lands, so PE work overlaps the produce stream; the
    window, not the PE, is critical. Chase drains run on ACT (it can read
    PSUM) because at the chase->steady boundary the DVE still has the
    wq-stream tail queued and drains behind it would stall PSUM recycling.
  - steady state (zero PE gaps measured, 3.49us/cell = MM-issue floor):
    per iter t: x_load(t+2) on scalar ring, full x-prep chain for t+1,
    4x16 matmuls, y store. xqT ring of 2, ldx ring of 2, 1-tile lookahead.
  - y is stored bf16 (host upcasts), rel err 3.0e-3 total; the last tile
    stores per-quarter to shorten the tail.
Dead ends (measured): sampled/bf16 w_scale (1.7-4e-2 err), sharded pass-1 +
AllReduce (~80us collective), fp8 DoubleRow x_q (exact hi/lo needs 2x
virtual MACs = breakeven minus overheads; single-pass fp8 approx 2.3e-2 >
budget, half-fp8 1.7e-2 too close), bf16 x pipeline via cast-DMA (works,
7e-3 err, but SWDGE loads starve beside the W stream), bf16 magic-192 W
round (double-rounding flips), gpsimd tensor ops (29us each), per-quarter
y stores for all tiles, high_priority on the w_scale chain or gpsimd-ring
x0/x1 loads (both regressed: scheduler butterfly effects, +5 to +55us).
"""

from contextlib import ExitStack

import numpy as np

import concourse.bass as bass
import concourse.tile as tile
from concourse import bacc, bass_isa, mybir
from concourse.bass import ds, ts
from concourse.bass_utils import run_bass_kernel_spmd

F32 = mybir.dt.float32
BF16 = mybir.dt.bfloat16
FP8 = mybir.dt.float8e4
AF = mybir.ActivationFunctionType
OP = mybir.AluOpType
AX = mybir.AxisListType

B, S, D_IN, D_OUT = 4, 4096, 2048, 2048
N_CORES = 8
TOK = B * S                # 16384 tokens
TPC = TOK // N_CORES       # 2048 tokens per core
NT = TPC // 128            # 16 token tiles per core
NB = D_IN // 128           # 16 contraction (k) blocks
NO = D_OUT // 512          # 4 output column blocks
HALF = D_OUT // 2          # 1024
CM = 12582912.0            # 1.5 * 2^23: fp32 RNE rounding magic
QMAX = 127.0

KNOBS = {
    "ldx_bufs": 2,
    "xq_bufs": 1,
    "t1_bufs": 2,
    "xqt_bufs": 2,
    "ys_bufs": 2,
    "psum_bufs": 8,
    "clamp_engine": "vector",
    "w_chunks": 8,
}

_CACHE = {}


def _emit(tc: tile.TileContext, x_d: bass.AP, w_d: bass.AP, y_d: bass.AP):
    nc = tc.nc
    clamp_eng = {"gpsimd": nc.gpsimd, "vector": nc.vector}[KNOBS["clamp_engine"]]
    with ExitStack() as ctx:
        wres = ctx.enter_context(tc.tile_pool(name="wres", bufs=1))
        wqp = ctx.enter_context(tc.tile_pool(name="wqp", bufs=1))
        ldx = ctx.enter_context(tc.tile_pool(name="ldx", bufs=KNOBS["ldx_bufs"]))
        xqp = ctx.enter_context(tc.tile_pool(name="xqp", bufs=KNOBS["xq_bufs"]))
        xqtp = ctx.enter_context(tc.tile_pool(name="xqtp", bufs=KNOBS["xqt_bufs"]))
        ysp = ctx.enter_context(tc.tile_pool(name="ysp", bufs=KNOBS["ys_bufs"]))
        t1p = ctx.enter_context(tc.tile_pool(name="t1p", bufs=KNOBS["t1_bufs"]))
        stats = ctx.enter_context(tc.tile_pool(name="stats", bufs=4))
        consts = ctx.enter_context(tc.tile_pool(name="consts", bufs=1))
        psum = ctx.enter_context(
            tc.tile_pool(name="psum", bufs=KNOBS["psum_bufs"], space=bass.MemorySpace.PSUM)
        )

        # ---- everything loads on the ONE sync/HWDGE ring, and the ring's
        # FIFO order IS the schedule: x0, x1 first (needed for prep during
        # pass-1), then the whole W stream, then x2, x3, ... behind it.
        # x loads are plain f32 (a cast-DMA must go via the SWDGE/gpsimd
        # ring, which only gets ~70-100 GB/s while the W stream runs and
        # made every downstream x op unpredictably late).
        #
        # W: the host pre-tiles W to [128, 16*2048] (partition p holds
        # wT[j*128+p, :] for all j), so W loads as a few BIG DMAs into one
        # fully-resident tile. Few DMAs matter: Tile has ~9 DMA
        # completion-sem lanes, so a 10th+ in-flight dma_start stalls its
        # issue on an earlier DMA's full completion (measured: 16x1MB
        # pushed the last W arrival from ~54us to ~71us).
        xtiles = {}

        def x_load(t, eng=None):
            xt = ldx.tile([128, D_IN], F32, tag="ldx", name=f"x{t}")
            (eng or nc.sync).dma_start(xt, x_d[ts(t, 128), :])
            xtiles[t] = xt

        x_load(0)
        x_load(1)

        NCH = KNOBS["w_chunks"]
        CHW = (NB // NCH) * D_OUT            # chunk width in f32 columns
        W1 = wres.tile([128, NB * D_OUT], F32, tag="W1", name="W1")
        for ch in range(NCH):
            nc.sync.dma_start(
                W1[:, ds(ch * CHW, CHW)], w_d[:, ds(ch * CHW, CHW)]
            )
        wt = [W1[:, ds(b * D_OUT, D_OUT)] for b in range(NB)]

        cpos = consts.tile([128, 1], F32, tag="cpos")
        nc.vector.memset(cpos, CM)
        czero = consts.tile([128, 1], F32, tag="czero")
        nc.vector.memset(czero, 0.0)
        # dummy activation on a ready constant: triggers the one-time
        # ACT_TABLE_LOAD during DMA warmup instead of on the critical chain
        warm = stats.tile([128, 1], F32, tag="warm")
        nc.scalar.activation(warm, czero, AF.Abs, bias=czero)

        # pass-1 abs-sums: even j on DVE (reduce, non-clobbering), odd j on
        # ACT (Abs -> throwaway bf16 scratch in the idle ys ring, accum_out
        # catches the column sum) so neither engine gates the W stream.
        wsumsD = stats.tile([128, NB // 2], F32, tag="wsumsD")
        wsumsA = stats.tile([128, NB // 2], F32, tag="wsumsA")

        def pass1(j):
            if j % 2 == 0:
                nc.vector.reduce_sum(
                    wsumsD[:, ds(j // 2, 1)], wt[j], axis=AX.X,
                    apply_absolute_value=True,
                )
            else:
                scr = ysp.tile([128, D_OUT], BF16, tag="ys", name=f"p1scr{j}")
                nc.scalar.activation(
                    scr, wt[j], AF.Abs, bias=czero,
                    accum_out=wsumsA[:, ds(j // 2, 1)],
                )

        xscales = {}

        def x_stats(t):
            a = stats.tile([128, 1], F32, tag="xa", name=f"xa{t}")
            nc.vector.reduce_max(a, xtiles[t], axis=AX.X, apply_absolute_value=True)
            nc.vector.tensor_scalar(a, a, 1e-8, None, OP.max)
            r0 = stats.tile([128, 1], F32, tag="xr0", name=f"xr0{t}")
            nc.vector.reciprocal(r0, a)
            ntt = stats.tile([128, 1], F32, tag="xntt", name=f"xntt{t}")
            nc.vector.tensor_mul(ntt, a, r0)
            nc.vector.tensor_scalar(ntt, ntt, -1.0, 2.0, OP.mult, OP.add)
            s = stats.tile([128, 1], F32, tag="xs", name=f"xs{t}")
            nc.vector.tensor_mul(s, r0, ntt)
            nc.vector.tensor_scalar(s, s, QMAX, None, OP.mult)  # 127/a
            xscales[t] = (a, s)

        xqts = {}

        def x_quant(t):
            a, s = xscales[t]
            xt = xtiles.pop(t)
            xq = xqp.tile([128, D_IN], BF16, tag="xq", name=f"xq{t}")
            for h in range(2):
                t1 = t1p.tile([128, HALF], F32, tag="t1", name=f"xt1_{t}_{h}")
                nc.scalar.activation(
                    t1, xt[:, ds(h * HALF, HALF)], AF.Identity, bias=cpos, scale=s
                )
                nc.vector.tensor_scalar(
                    xq[:, ds(h * HALF, HALF)], t1, -CM, None, OP.add
                )
            xqT = xqtp.tile([128, NB, 128], BF16, tag="xqT", name=f"xqT{t}")
            nc.sync.dma_start(xqT, xq, transpose=True)
            xqts[t] = xqT

        souts = {}

        def x_sout(t):
            a, _ = xscales[t]
            so = stats.tile([128, 1], F32, tag="xso", name=f"xso{t}")
            nc.vector.tensor_scalar(so, a, ws127, None, OP.mult)
            souts[t] = so

        # One elevated-priority block holds pass-1 AND the x0/x1/x2-stats
        # prep in the static order we want (the scheduler's per-engine
        # order tracks priority; ops left outside run after the whole
        # stream and land as insertions on the w_scale/wq critical path).
        # offset=8 (not a full reset to 0): a reset collides these ops
        # with the W-chunk dma_start priorities and scrambles the sync
        # ring order (measured +6us on the chase start).
        with tc.high_priority(offset=8):
            pass1(0)
            pass1(1)
            pass1(2)
            x_stats(0)
            x_quant(0)
            pass1(3)
            pass1(4)
            pass1(5)
            x_stats(1)
            x_quant(1)
            for j in range(6, NB):
                pass1(j)
            x_load(2)
            x_load(3)
            x_stats(2)

        # ---- w_scale ----
        wsD = stats.tile([128, 1], F32, tag="wsD")
        nc.vector.reduce_sum(wsD, wsumsD, axis=AX.X)
        wsA = stats.tile([128, 1], F32, tag="wsA")
        nc.vector.reduce_sum(wsA, wsumsA, axis=AX.X)
        wsum_p = stats.tile([128, 1], F32, tag="wsp")
        nc.vector.tensor_add(wsum_p, wsD, wsA)
        wsum_all = stats.tile([128, 1], F32, tag="wsa")
        nc.gpsimd.partition_all_reduce(wsum_all, wsum_p, 128, bass_isa.ReduceOp.add)
        wscale = consts.tile([128, 1], F32, tag="wscale")
        nc.vector.tensor_scalar(
            wscale, wsum_all, 1.0 / (D_OUT * D_IN), 1e-6, OP.mult, OP.max
        )
        r0 = stats.tile([128, 1], F32, tag="wr0")
        nc.vector.reciprocal(r0, wscale)
        ntt = stats.tile([128, 1], F32, tag="wntt")
        nc.vector.tensor_mul(ntt, wscale, r0)
        nc.vector.tensor_scalar(ntt, ntt, -1.0, 2.0, OP.mult, OP.add)
        rws = consts.tile([128, 1], F32, tag="rws")
        nc.vector.tensor_mul(rws, r0, ntt)
        ws127 = consts.tile([128, 1], F32, tag="ws127")
        nc.vector.tensor_scalar(ws127, wscale, 1.0 / QMAX, None, OP.mult)
        x_sout(0)
        x_sout(1)

        # ---- W quantize stream + PE chase-ramp ----
        wq = [
            wqp.tile([128, D_OUT], FP8, tag=f"wq{b}", name=f"wq{b}")
            for b in range(NB)
        ]

        def w_quant(b):
            for h in range(2):
                t1 = t1p.tile([128, HALF], F32, tag="t1", name=f"wt1_{b}_{h}")
                nc.scalar.activation(
                    t1, wt[b][:, ds(h * HALF, HALF)], AF.Identity,
                    bias=cpos, scale=rws,
                )
                nc.vector.tensor_scalar(
                    wq[b][:, ds(h * HALF, HALF)], t1, -CM, None, OP.add
                )
            clamp_eng.tensor_scalar(wq[b], wq[b], -1.0, 1.0, OP.max, OP.min)

        # The wq stream is high priority (it gates the PE chase); x tiles
        # 2,3 prep threads through its engine gaps (ACT/DVE have ~0.5us/b
        # of slack vs the chase) so their xqT are ready before the chase
        # ends and the steady state starts without a stall.
        with tc.high_priority():
            for b in range(NB):
                w_quant(b)
        x_quant(2)
        x_sout(2)

        # 8 PSUM cells (tiles 0-1 x col-blocks 0-3) accumulate each b as its
        # wq lands; PE consumes at ~1.7us/b vs ~2.2us/b quantize rate.
        chase = [(t, no) for t in range(2) for no in range(NO)]
        pss = {}
        for c, (t, no) in enumerate(chase):
            pss[c] = psum.tile([128, 512], F32, tag="ps", name=f"cps{c}")
        for b in range(NB):
            for c, (t, no) in enumerate(chase):
                nc.tensor.matmul(
                    pss[c],
                    xqts[t][:, b, :],
                    wq[b][:, ds(no * 512, 512)],
                    start=(b == 0),
                    stop=(b == NB - 1),
                )

        ys = {}

        def y_tile(t):
            if t not in ys:
                ys[t] = ysp.tile([128, D_OUT], BF16, tag="ys", name=f"ys{t}")
            return ys[t]

        # chase drains run on ACT (it can read PSUM): at the chase->steady
        # boundary the DVE still has the wq-stream tail queued, and drains
        # stuck behind it would stall PSUM recycling for the first steady
        # cells.
        for c, (t, no) in enumerate(chase):
            nc.scalar.activation(
                y_tile(t)[:, ds(no * 512, 512)], pss[c], AF.Identity,
                bias=czero, scale=souts[t],
            )
        del pss

        def y_store(t):
            nc.sync.dma_start(y_d[ts(t, 128), :], ys.pop(t))
            del xqts[t]

        y_store(0)
        y_store(1)

        # ---- steady state: x-prep two tiles ahead ----
        def cell(no, t, store_quarter=False):
            ps = psum.tile([128, 512], F32, tag="ps")
            xqT = xqts[t]
            for b in range(NB):
                nc.tensor.matmul(
                    ps,
                    xqT[:, b, :],
                    wq[b][:, ds(no * 512, 512)],
                    start=(b == 0),
                    stop=(b == NB - 1),
                )
            nc.vector.tensor_scalar(
                y_tile(t)[:, ds(no * 512, 512)], ps, souts[t], None, OP.mult
            )
            if store_quarter:
                nc.sync.dma_start(
                    y_d[ts(t, 128), ds(no * 512, 512)],
                    ys[t][:, ds(no * 512, 512)],
                )

        # steady x loads ride the scalar HWDGE ring: the sync ring then
        # carries only transposes + y stores, so a prep transpose is never
        # serialized behind a 1MB in-flight x load (DMA-transpose is
        # ordered against prior DMAs on its ring).
        for t in range(2, NT):
            last = t == NT - 1
            if t + 2 < NT:
                x_load(t + 2, eng=nc.scalar)
            if t + 1 < NT:
                x_stats(t + 1)
                x_quant(t + 1)
                x_sout(t + 1)
            for no in range(NO):
                cell(no, t, store_quarter=last)
            if last:
                ys.pop(t)
                del xqts[t]
            else:
                y_store(t)


def _build():
    key = tuple(sorted((k, str(v)) for k, v in KNOBS.items()))
    if key in _CACHE:
        return _CACHE[key]
    nc = bacc.Bacc(
        "TRN2", target_bir_lowering=False, debug=False, num_devices=N_CORES
    )
    x_d = nc.dram_tensor("x", [TPC, D_IN], F32, kind="ExternalInput").ap()
    # w is fed pre-tiled by the host: w2[p, b*2048 + c] = W[c, b*128 + p]
    w_d = nc.dram_tensor("w", [128, NB * D_OUT], F32, kind="ExternalInput").ap()
    y_d = nc.dram_tensor("y", [TPC, D_OUT], BF16, kind="ExternalOutput").ap()
    with tile.TileContext(nc) as tc:
        _emit(tc, x_d, w_d, y_d)
    nc.compile()
    _CACHE[key] = nc
    return nc


_last_result = None  # BassKernelResults of the most recent run (for profiling)


def kernel(x: np.ndarray, weight: np.ndarray, trace: bool = False) -> np.ndarray:
    global _last_result
    nc = _build()
    xf = np.ascontiguousarray(x.reshape(TOK, D_IN), dtype=np.float32)
    wT2 = np.ascontiguousarray(
        weight.T.astype(np.float32)
        .reshape(NB, 128, D_OUT).transpose(1, 0, 2).reshape(128, NB * D_OUT)
    )
    in_maps = [
        {"x": xf[c * TPC:(c + 1) * TPC], "w": wT2}
        for c in range(N_CORES)
    ]
    res = run_bass_kernel_spmd(nc, in_maps, list(range(N_CORES)), trace=trace)
    _last_result = res
    y = np.concatenate(
        [np.asarray(res.results[c]["y"]) for c in range(N_CORES)], axis=0
    )
    return y.reshape(B, S, D_OUT).astype(np.float32)


# revision 36
# speedup vs baseline: 1.0145x; 1.0145x over previous
"""BitLinear (ternary-weight + 8-bit-activation quantized matmul) on 8 TRN2 cores.

Strategy: data-parallel over tokens. Each core gets 2048 of the 16384 tokens
plus the full weight matrix, computes the whole BitLinear forward for its
token shard on device, and the host concatenates the shards.

Math (must match the jax reference):
  w_scale = max(mean(|W|), 1e-6)                       (scalar)
  w_q     = clip(round(W / w_scale), -1, 1)            (ternary)
  a       = clip(max_i |x|, 1e-8, inf)                 (per token)
  x_q     = clip(round(x * 127 / a), -127, 127)        (8-bit ints)
  y       = (x_q @ w_q^T) * w_scale * a / 127

Final schedule (~318-322us measured vs 394us v1 baseline; roofline: 221us
bf16 GEMM + ~49us W stream + ~48us quantize window + fixed ~18us of
framework preamble/teardown -> ~306us floor):
  - w_scale is extremely sensitive (2e-4 rel deviation flips ternary weights
    near .5 boundaries -> 3e-2 err), so pass 1 must abs-sum the full fp32 W.
    W is FULLY RESIDENT in fp32 (128KB of ~207KB/partition SBUF): zero
    re-read; the host pre-tiles it to [128, 16*2048] so it streams once as
    8 x 2MB DMAs. Few DMAs matter: Tile has ~9 DMA completion-sem lanes and
    a 10th+ in-flight dma_start stalls its *issue* on an earlier DMA's full
    completion (16x1MB pushed the last W arrival from ~54us to ~71us).
  - ALL bulk transfers ride the sync/HWDGE ring and its FIFO order IS the
    schedule: x0, x1 first (prep runs during pass-1), then W, then x2, x3
    behind it. Steady-state x loads ride the scalar HWDGE ring so a prep
    DMA-transpose is never serialized behind an in-flight 1MB x load.
    SWDGE/gpsimd DMAs are useless here: compute ops on gpsimd take ~29us
    per [128,2048] tile and cast-DMAs only get ~70-100 GB/s beside the W
    stream, arriving late and (via the static scheduler's optimistic DMA
    model) head-of-line blocking the DVE queue.
  - pass-1 abs-sums alternate DVE reduce_sum (even j, non-clobbering) and
    ACT Abs->bf16 scratch in the idle ys ring with accum_out (odd j;
    bf16-rounded |W| sums are random-error ~1e-6 rel over 4.2M elements)
    inside tc.high_priority() so stray x-prep ops cannot park ahead of
    them on an engine; w_scale lands ~3us after the last W chunk.
  - rounding is the fp32 magic-number trick (+1.5*2^23, exact RNE; a bf16
    magic of +192 double-rounds and flips ~16 weights at the .5 boundary =
    up to ~1.8e-2 err, rejected). Quantize runs on [128,1024] halves
    through a shared 2-buf t1 pool. W: ACT magic -> DVE subtract into fp8
    (ints <= 8 exact in e4m3) -> full-row in-place fp8 clamp on DVE. x:
    ACT magic -> DVE subtract to bf16, no clamp needed (|x*127/a| <= 127
    by construction). x stays f32 end-to-end (3.0e-3 total err).
  - the quantize window (~48us) is BOTH-engine-saturated (ACT ~2.5us/b,
    DVE ~2.6us/b incl clamp, + x2 prep + w_scale chain) - measured at its
    floor; 3 elementwise passes over W are irreducible with 2 engines
    (ACT cannot clamp: no min/clip activation function).
  - GEMM ramp: 8 PSUM cells (tiles 0-1 x 4 col-blocks) accumulate each b
    the moment its wq lands, so PE work overlaps the produce stream; the
    window, not the PE, is critical. Chase drains run on ACT (it can read
    PSUM) because at the chase->steady boundary the DVE still has the
    wq-stream tail queued and drains behind it would stall PSUM recycling.
  - steady state (zero PE gaps measured, 3.49us/cell = MM-issue floor):
    per iter t: x_load(t+2) on scalar ring, full x-prep chain for t+1,
    4x16 matmuls, y store. xqT ring of 2, ldx ring of 2, 1-tile lookahead.
  - y is stored bf16 (host upcasts), rel err 3.0e-3 total; the last tile
    stores per-quarter to shorten the tail.
Dead ends (measured): sampled/bf16 w_scale (1.7-4e-2 err), sharded pass-1 +
AllReduce (~80us collective), fp8 DoubleRow x_q (exact hi/lo needs 2x
virtual MACs = breakeven minus overheads; single-pass fp8 approx 2.3e-2 >
budget, half-fp8 1.7e-2 too close), bf16 x pipeline via cast-DMA (works,
7e-3 err, but SWDGE loads starve beside the W stream), bf16 magic-192 W
round (double-rounding flips), gpsimd tensor ops (29us each), per-quarter
y stores for all tiles, high_priority on the w_scale chain or gpsimd-ring
x0/x1 loads (both regressed: scheduler butterfly effects, +5 to +55us).
"""

from contextlib import ExitStack

import numpy as np

import concourse.bass as bass
import concourse.tile as tile
from concourse import bacc, bass_isa, mybir
from concourse.bass import ds, ts
from concourse.bass_utils import run_bass_kernel_spmd

F32 = mybir.dt.float32
BF16 = mybir.dt.bfloat16
FP8 = mybir.dt.float8e4
AF = mybir.ActivationFunctionType
OP = mybir.AluOpType
AX = mybir.AxisListType

B, S, D_IN, D_OUT = 4, 4096, 2048, 2048
N_CORES = 8
TOK = B * S                # 16384 tokens
TPC = TOK // N_CORES       # 2048 tokens per core
NT = TPC // 128            # 16 token tiles per core
NB = D_IN // 128           # 16 contraction (k) blocks
NO = D_OUT // 512          # 4 output column blocks
HALF = D_OUT // 2          # 1024
CM = 12582912.0            # 1.5 * 2^23: fp32 RNE rounding magic
QMAX = 127.0

KNOBS = {
    "ldx_bufs": 2,
    "xq_bufs": 1,
    "t1_bufs": 2,
    "xqt_bufs": 2,
    "ys_bufs": 2,
    "psum_bufs": 8,
    "clamp_engine": "vector",
    "w_chunks": 4,
}

_CACHE = {}


def _emit(tc: tile.TileContext, x_d: bass.AP, w_d: bass.AP, y_d: bass.AP):
    nc = tc.nc
    clamp_eng = {"gpsimd": nc.gpsimd, "vector": nc.vector}[KNOBS["clamp_engine"]]
    with ExitStack() as ctx:
        wres = ctx.enter_context(tc.tile_pool(name="wres", bufs=1))
        wqp = ctx.enter_context(tc.tile_pool(name="wqp", bufs=1))
        ldx = ctx.enter_context(tc.tile_pool(name="ldx", bufs=KNOBS["ldx_bufs"]))
        xqp = ctx.enter_context(tc.tile_pool(name="xqp", bufs=KNOBS["xq_bufs"]))
        xqtp = ctx.enter_context(tc.tile_pool(name="xqtp", bufs=KNOBS["xqt_bufs"]))
        ysp = ctx.enter_context(tc.tile_pool(name="ysp", bufs=KNOBS["ys_bufs"]))
        t1p = ctx.enter_context(tc.tile_pool(name="t1p", bufs=KNOBS["t1_bufs"]))
        stats = ctx.enter_context(tc.tile_pool(name="stats", bufs=4))
        consts = ctx.enter_context(tc.tile_pool(name="consts", bufs=1))
        psum = ctx.enter_context(
            tc.tile_pool(name="psum", bufs=KNOBS["psum_bufs"], space=bass.MemorySpace.PSUM)
        )

        # ---- everything loads on the ONE sync/HWDGE ring, and the ring's
        # FIFO order IS the schedule: x0, x1 first (needed for prep during
        # pass-1), then the whole W stream, then x2, x3, ... behind it.
        # x loads are plain f32 (a cast-DMA must go via the SWDGE/gpsimd
        # ring, which only gets ~70-100 GB/s while the W stream runs and
        # made every downstream x op unpredictably late).
        #
        # W: the host pre-tiles W to [128, 16*2048] (partition p holds
        # wT[j*128+p, :] for all j), so W loads as a few BIG DMAs into one
        # fully-resident tile. Few DMAs matter: Tile has ~9 DMA
        # completion-sem lanes, so a 10th+ in-flight dma_start stalls its
        # issue on an earlier DMA's full completion (measured: 16x1MB
        # pushed the last W arrival from ~54us to ~71us).
        xtiles = {}

        def x_load(t, eng=None):
            xt = ldx.tile([128, D_IN], F32, tag="ldx", name=f"x{t}")
            (eng or nc.sync).dma_start(xt, x_d[ts(t, 128), :])
            xtiles[t] = xt

        x_load(0)
        x_load(1)

        NCH = KNOBS["w_chunks"]
        CHW = (NB // NCH) * D_OUT            # chunk width in f32 columns
        W1 = wres.tile([128, NB * D_OUT], F32, tag="W1", name="W1")
        for ch in range(NCH):
            nc.sync.dma_start(
                W1[:, ds(ch * CHW, CHW)], w_d[:, ds(ch * CHW, CHW)]
            )
        wt = [W1[:, ds(b * D_OUT, D_OUT)] for b in range(NB)]

        cpos = consts.tile([128, 1], F32, tag="cpos")
        nc.vector.memset(cpos, CM)
        czero = consts.tile([128, 1], F32, tag="czero")
        nc.vector.memset(czero, 0.0)
        # dummy activation on a ready constant: triggers the one-time
        # ACT_TABLE_LOAD during DMA warmup instead of on the critical chain
        warm = stats.tile([128, 1], F32, tag="warm")
        nc.scalar.activation(warm, czero, AF.Abs, bias=czero)

        # pass-1 abs-sums: even j on DVE (reduce, non-clobbering), odd j on
        # ACT (Abs -> throwaway bf16 scratch in the idle ys ring, accum_out
        # catches the column sum) so neither engine gates the W stream.
        wsumsD = stats.tile([128, NB // 2], F32, tag="wsumsD")
        wsumsA = stats.tile([128, NB // 2], F32, tag="wsumsA")

        def pass1(j):
            if j % 2 == 0:
                nc.vector.reduce_sum(
                    wsumsD[:, ds(j // 2, 1)], wt[j], axis=AX.X,
                    apply_absolute_value=True,
                )
            else:
                scr = ysp.tile([128, D_OUT], BF16, tag="ys", name=f"p1scr{j}")
                nc.scalar.activation(
                    scr, wt[j], AF.Abs, bias=czero,
                    accum_out=wsumsA[:, ds(j // 2, 1)],
                )

        xscales = {}

        def x_stats(t):
            a = stats.tile([128, 1], F32, tag="xa", name=f"xa{t}")
            nc.vector.reduce_max(a, xtiles[t], axis=AX.X, apply_absolute_value=True)
            nc.vector.tensor_scalar(a, a, 1e-8, None, OP.max)
            r0 = stats.tile([128, 1], F32, tag="xr0", name=f"xr0{t}")
            nc.vector.reciprocal(r0, a)
            ntt = stats.tile([128, 1], F32, tag="xntt", name=f"xntt{t}")
            nc.vector.tensor_mul(ntt, a, r0)
            nc.vector.tensor_scalar(ntt, ntt, -1.0, 2.0, OP.mult, OP.add)
            s = stats.tile([128, 1], F32, tag="xs", name=f"xs{t}")
            nc.vector.tensor_mul(s, r0, ntt)
            nc.vector.tensor_scalar(s, s, QMAX, None, OP.mult)  # 127/a
            xscales[t] = (a, s)

        xqts = {}

        def x_quant(t):
            a, s = xscales[t]
            xt = xtiles.pop(t)
            xq = xqp.tile([128, D_IN], BF16, tag="xq", name=f"xq{t}")
            for h in range(2):
                t1 = t1p.tile([128, HALF], F32, tag="t1", name=f"xt1_{t}_{h}")
                nc.scalar.activation(
                    t1, xt[:, ds(h * HALF, HALF)], AF.Identity, bias=cpos, scale=s
                )
                nc.vector.tensor_scalar(
                    xq[:, ds(h * HALF, HALF)], t1, -CM, None, OP.add
                )
            xqT = xqtp.tile([128, NB, 128], BF16, tag="xqT", name=f"xqT{t}")
            nc.sync.dma_start(xqT, xq, transpose=True)
            xqts[t] = xqT

        souts = {}

        def x_sout(t):
            a, _ = xscales[t]
            so = stats.tile([128, 1], F32, tag="xso", name=f"xso{t}")
            nc.vector.tensor_scalar(so, a, ws127, None, OP.mult)
            souts[t] = so

        # pass-1 sums get high priority so the scheduler never parks them
        # behind the x chains. (Interleaving the x prep INSIDE this block
        # was tried twice -- priority reset AND offset=8 -- and both made
        # the schedule noisier/slower: the scheduler is at a local optimum.)
        with tc.high_priority():
            for j in range(NB):
                pass1(j)
        x_stats(0)
        x_quant(0)
        x_stats(1)
        x_quant(1)

        # ---- w_scale ----
        wsD = stats.tile([128, 1], F32, tag="wsD")
        nc.vector.reduce_sum(wsD, wsumsD, axis=AX.X)
        wsA = stats.tile([128, 1], F32, tag="wsA")
        nc.vector.reduce_sum(wsA, wsumsA, axis=AX.X)
        wsum_p = stats.tile([128, 1], F32, tag="wsp")
        nc.vector.tensor_add(wsum_p, wsD, wsA)
        wsum_all = stats.tile([128, 1], F32, tag="wsa")
        nc.gpsimd.partition_all_reduce(wsum_all, wsum_p, 128, bass_isa.ReduceOp.add)
        wscale = consts.tile([128, 1], F32, tag="wscale")
        nc.vector.tensor_scalar(
            wscale, wsum_all, 1.0 / (D_OUT * D_IN), 1e-6, OP.mult, OP.max
        )
        r0 = stats.tile([128, 1], F32, tag="wr0")
        nc.vector.reciprocal(r0, wscale)
        ntt = stats.tile([128, 1], F32, tag="wntt")
        nc.vector.tensor_mul(ntt, wscale, r0)
        nc.vector.tensor_scalar(ntt, ntt, -1.0, 2.0, OP.mult, OP.add)
        rws = consts.tile([128, 1], F32, tag="rws")
        nc.vector.tensor_mul(rws, r0, ntt)
        ws127 = consts.tile([128, 1], F32, tag="ws127")
        nc.vector.tensor_scalar(ws127, wscale, 1.0 / QMAX, None, OP.mult)
        x_sout(0)
        x_sout(1)

        # x tiles 2,3: issue is gated on ldx slots freeing (~15-20us), so
        # these land on the sync ring BEHIND the W chunks and complete
        # right after the stream -- in time for prep during/after the chase.
        x_load(2)
        x_load(3)

        # ---- W quantize stream + PE chase-ramp ----
        wq = [
            wqp.tile([128, D_OUT], FP8, tag=f"wq{b}", name=f"wq{b}")
            for b in range(NB)
        ]

        def w_quant(b):
            for h in range(2):
                t1 = t1p.tile([128, HALF], F32, tag="t1", name=f"wt1_{b}_{h}")
                nc.scalar.activation(
                    t1, wt[b][:, ds(h * HALF, HALF)], AF.Identity,
                    bias=cpos, scale=rws,
                )
                nc.vector.tensor_scalar(
                    wq[b][:, ds(h * HALF, HALF)], t1, -CM, None, OP.add
                )
            clamp_eng.tensor_scalar(wq[b], wq[b], -1.0, 1.0, OP.max, OP.min)

        # The wq stream is high priority (it gates the PE chase); x tiles
        # 2,3 prep threads through its engine gaps (ACT/DVE have ~0.5us/b
        # of slack vs the chase) so their xqT are ready before the chase
        # ends and the steady state starts without a stall.
        with tc.high_priority():
            for b in range(NB):
                w_quant(b)
        x_stats(2)
        x_quant(2)
        x_sout(2)

        # 8 PSUM cells (tiles 0-1 x col-blocks 0-3) accumulate each b as its
        # wq lands; PE consumes at ~1.7us/b vs ~2.2us/b quantize rate.
        chase = [(t, no) for t in range(2) for no in range(NO)]
        pss = {}
        for c, (t, no) in enumerate(chase):
            pss[c] = psum.tile([128, 512], F32, tag="ps", name=f"cps{c}")
        for b in range(NB):
            for c, (t, no) in enumerate(chase):
                nc.tensor.matmul(
                    pss[c],
                    xqts[t][:, b, :],
                    wq[b][:, ds(no * 512, 512)],
                    start=(b == 0),
                    stop=(b == NB - 1),
                )

        ys = {}

        def y_tile(t):
            if t not in ys:
                ys[t] = ysp.tile([128, D_OUT], BF16, tag="ys", name=f"ys{t}")
            return ys[t]

        # chase drains run on ACT (it can read PSUM): at the chase->steady
        # boundary the DVE still has the wq-stream tail queued, and drains
        # stuck behind it would stall PSUM recycling for the first steady
        # cells.
        for c, (t, no) in enumerate(chase):
            nc.scalar.activation(
                y_tile(t)[:, ds(no * 512, 512)], pss[c], AF.Identity,
                bias=czero, scale=souts[t],
            )
        del pss

        def y_store(t):
            nc.sync.dma_start(y_d[ts(t, 128), :], ys.pop(t))
            del xqts[t]

        y_store(0)
        y_store(1)

        # ---- steady state: x-prep two tiles ahead ----
        def cell(no, t, store_quarter=False):
            ps = psum.tile([128, 512], F32, tag="ps")
            xqT = xqts[t]
            for b in range(NB):
                nc.tensor.matmul(
                    ps,
                    xqT[:, b, :],
                    wq[b][:, ds(no * 512, 512)],
                    start=(b == 0),
                    stop=(b == NB - 1),
                )
            nc.vector.tensor_scalar(
                y_tile(t)[:, ds(no * 512, 512)], ps, souts[t], None, OP.mult
            )
            if store_quarter:
                nc.sync.dma_start(
                    y_d[ts(t, 128), ds(no * 512, 512)],
                    ys[t][:, ds(no * 512, 512)],
                )

        # steady x loads ride the scalar HWDGE ring: the sync ring then
        # carries only transposes + y stores, so a prep transpose is never
        # serialized behind a 1MB in-flight x load (DMA-transpose is
        # ordered against prior DMAs on its ring).
        for t in range(2, NT):
            last = t == NT - 1
            if t + 2 < NT:
                x_load(t + 2, eng=nc.scalar)
            if t + 1 < NT:
                x_stats(t + 1)
                x_quant(t + 1)
                x_sout(t + 1)
            for no in range(NO):
                cell(no, t, store_quarter=last)
            if last:
                ys.pop(t)
                del xqts[t]
            else:
                y_store(t)


def _build():
    key = tuple(sorted((k, str(v)) for k, v in KNOBS.items()))
    if key in _CACHE:
        return _CACHE[key]
    nc = bacc.Bacc(
        "TRN2", target_bir_lowering=False, debug=False, num_devices=N_CORES
    )
    x_d = nc.dram_tensor("x", [TPC, D_IN], F32, kind="ExternalInput").ap()
    # w is fed pre-tiled by the host: w2[p, b*2048 + c] = W[c, b*128 + p]
    w_d = nc.dram_tensor("w", [128, NB * D_OUT], F32, kind="ExternalInput").ap()
    y_d = nc.dram_tensor("y", [TPC, D_OUT], BF16, kind="ExternalOutput").ap()
    with tile.TileContext(nc) as tc:
        _emit(tc, x_d, w_d, y_d)
    nc.compile()
    _CACHE[key] = nc
    return nc


_last_result = None  # BassKernelResults of the most recent run (for profiling)


def kernel(x: np.ndarray, weight: np.ndarray, trace: bool = False) -> np.ndarray:
    global _last_result
    nc = _build()
    xf = np.ascontiguousarray(x.reshape(TOK, D_IN), dtype=np.float32)
    wT2 = np.ascontiguousarray(
        weight.T.astype(np.float32)
        .reshape(NB, 128, D_OUT).transpose(1, 0, 2).reshape(128, NB * D_OUT)
    )
    in_maps = [
        {"x": xf[c * TPC:(c + 1) * TPC], "w": wT2}
        for c in range(N_CORES)
    ]
    res = run_bass_kernel_spmd(nc, in_maps, list(range(N_CORES)), trace=trace)
    _last_result = res
    y = np.concatenate(
        [np.asarray(res.results[c]["y"]) for c in range(N_CORES)], axis=0
    )
    return y.reshape(B, S, D_OUT).astype(np.float32)


# revision 37
# speedup vs baseline: 1.0433x; 1.0285x over previous
"""BitLinear (ternary-weight + 8-bit-activation quantized matmul) on 8 TRN2 cores.

Strategy: data-parallel over tokens. Each core gets 2048 of the 16384 tokens
plus the full weight matrix, computes the whole BitLinear forward for its
token shard on device, and the host concatenates the shards.

Math (must match the jax reference):
  w_scale = max(mean(|W|), 1e-6)                       (scalar)
  w_q     = clip(round(W / w_scale), -1, 1)            (ternary)
  a       = clip(max_i |x|, 1e-8, inf)                 (per token)
  x_q     = clip(round(x * 127 / a), -127, 127)        (8-bit ints)
  y       = (x_q @ w_q^T) * w_scale * a / 127

Final schedule (~318-322us measured vs 394us v1 baseline; roofline: 221us
bf16 GEMM + ~49us W stream + ~48us quantize window + fixed ~18us of
framework preamble/teardown -> ~306us floor):
  - w_scale is extremely sensitive (2e-4 rel deviation flips ternary weights
    near .5 boundaries -> 3e-2 err), so pass 1 must abs-sum the full fp32 W.
    W is FULLY RESIDENT in fp32 (128KB of ~207KB/partition SBUF): zero
    re-read; the host pre-tiles it to [128, 16*2048] so it streams once as
    8 x 2MB DMAs. Few DMAs matter: Tile has ~9 DMA completion-sem lanes and
    a 10th+ in-flight dma_start stalls its *issue* on an earlier DMA's full
    completion (16x1MB pushed the last W arrival from ~54us to ~71us).
  - ALL bulk transfers ride the sync/HWDGE ring and its FIFO order IS the
    schedule: x0, x1 first (prep runs during pass-1), then W, then x2, x3
    behind it. Steady-state x loads ride the scalar HWDGE ring so a prep
    DMA-transpose is never serialized behind an in-flight 1MB x load.
    SWDGE/gpsimd DMAs are useless here: compute ops on gpsimd take ~29us
    per [128,2048] tile and cast-DMAs only get ~70-100 GB/s beside the W
    stream, arriving late and (via the static scheduler's optimistic DMA
    model) head-of-line blocking the DVE queue.
  - pass-1 abs-sums alternate DVE reduce_sum (even j, non-clobbering) and
    ACT Abs->bf16 scratch in the idle ys ring with accum_out (odd j;
    bf16-rounded |W| sums are random-error ~1e-6 rel over 4.2M elements)
    inside tc.high_priority() so stray x-prep ops cannot park ahead of
    them on an engine; w_scale lands ~3us after the last W chunk.
  - rounding is the fp32 magic-number trick (+1.5*2^23, exact RNE; a bf16
    magic of +192 double-rounds and flips ~16 weights at the .5 boundary =
    up to ~1.8e-2 err, rejected). Quantize runs on [128,1024] halves
    through a shared 2-buf t1 pool. W: ACT magic -> DVE subtract into fp8
    (ints <= 8 exact in e4m3) -> full-row in-place fp8 clamp on DVE. x:
    ACT magic -> DVE subtract to bf16, no clamp needed (|x*127/a| <= 127
    by construction). x stays f32 end-to-end (3.0e-3 total err).
  - the quantize window (~48us) is BOTH-engine-saturated (ACT ~2.5us/b,
    DVE ~2.6us/b incl clamp, + x2 prep + w_scale chain) - measured at its
    floor; 3 elementwise passes over W are irreducible with 2 engines
    (ACT cannot clamp: no min/clip activation function).
  - GEMM ramp: 8 PSUM cells (tiles 0-1 x 4 col-blocks) accumulate each b
    the moment its wq lands, so PE work overlaps the produce stream; the
    window, not the PE, is critical. Chase drains run on ACT (it can read
    PSUM) because at the chase->steady boundary the DVE still has the
    wq-stream tail queued and drains behind it would stall PSUM recycling.
  - steady state (zero PE gaps measured, 3.49us/cell = MM-issue floor):
    per iter t: x_load(t+2) on scalar ring, full x-prep chain for t+1,
    4x16 matmuls, y store. xqT ring of 2, ldx ring of 2, 1-tile lookahead.
  - y is stored bf16 (host upcasts), rel err 3.0e-3 total; the last tile
    stores per-quarter to shorten the tail.
Dead ends (measured): sampled/bf16 w_scale (1.7-4e-2 err), sharded pass-1 +
AllReduce (~80us collective), fp8 DoubleRow x_q (exact hi/lo needs 2x
virtual MACs = breakeven minus overheads; single-pass fp8 approx 2.3e-2 >
budget, half-fp8 1.7e-2 too close), bf16 x pipeline via cast-DMA (works,
7e-3 err, but SWDGE loads starve beside the W stream), bf16 magic-192 W
round (double-rounding flips), gpsimd tensor ops (29us each), per-quarter
y stores for all tiles, high_priority on the w_scale chain or gpsimd-ring
x0/x1 loads (both regressed: scheduler butterfly effects, +5 to +55us).
"""

from contextlib import ExitStack

import numpy as np

import concourse.bass as bass
import concourse.tile as tile
from concourse import bacc, bass_isa, mybir
from concourse.bass import ds, ts
from concourse.bass_utils import run_bass_kernel_spmd

F32 = mybir.dt.float32
BF16 = mybir.dt.bfloat16
FP8 = mybir.dt.float8e4
AF = mybir.ActivationFunctionType
OP = mybir.AluOpType
AX = mybir.AxisListType

B, S, D_IN, D_OUT = 4, 4096, 2048, 2048
N_CORES = 8
TOK = B * S                # 16384 tokens
TPC = TOK // N_CORES       # 2048 tokens per core
NT = TPC // 128            # 16 token tiles per core
NB = D_IN // 128           # 16 contraction (k) blocks
NO = D_OUT // 512          # 4 output column blocks
HALF = D_OUT // 2          # 1024
CM = 12582912.0            # 1.5 * 2^23: fp32 RNE rounding magic
QMAX = 127.0

KNOBS = {
    "ldx_bufs": 2,
    "xq_bufs": 1,
    "t1_bufs": 2,
    "xqt_bufs": 2,
    "ys_bufs": 2,
    "psum_bufs": 8,
    "clamp_engine": "vector",
    "w_chunks": 8,
}

_CACHE = {}


def _emit(tc: tile.TileContext, x_d: bass.AP, w_d: bass.AP, y_d: bass.AP):
    nc = tc.nc
    clamp_eng = {"gpsimd": nc.gpsimd, "vector": nc.vector}[KNOBS["clamp_engine"]]
    with ExitStack() as ctx:
        wres = ctx.enter_context(tc.tile_pool(name="wres", bufs=1))
        wqp = ctx.enter_context(tc.tile_pool(name="wqp", bufs=1))
        ldx = ctx.enter_context(tc.tile_pool(name="ldx", bufs=KNOBS["ldx_bufs"]))
        xqp = ctx.enter_context(tc.tile_pool(name="xqp", bufs=KNOBS["xq_bufs"]))
        xqtp = ctx.enter_context(tc.tile_pool(name="xqtp", bufs=KNOBS["xqt_bufs"]))
        ysp = ctx.enter_context(tc.tile_pool(name="ysp", bufs=KNOBS["ys_bufs"]))
        t1p = ctx.enter_context(tc.tile_pool(name="t1p", bufs=KNOBS["t1_bufs"]))
        stats = ctx.enter_context(tc.tile_pool(name="stats", bufs=4))
        consts = ctx.enter_context(tc.tile_pool(name="consts", bufs=1))
        psum = ctx.enter_context(
            tc.tile_pool(name="psum", bufs=KNOBS["psum_bufs"], space=bass.MemorySpace.PSUM)
        )

        # ---- everything loads on the ONE sync/HWDGE ring, and the ring's
        # FIFO order IS the schedule: x0, x1 first (needed for prep during
        # pass-1), then the whole W stream, then x2, x3, ... behind it.
        # x loads are plain f32 (a cast-DMA must go via the SWDGE/gpsimd
        # ring, which only gets ~70-100 GB/s while the W stream runs and
        # made every downstream x op unpredictably late).
        #
        # W: the host pre-tiles W to [128, 16*2048] (partition p holds
        # wT[j*128+p, :] for all j), so W loads as a few BIG DMAs into one
        # fully-resident tile. Few DMAs matter: Tile has ~9 DMA
        # completion-sem lanes, so a 10th+ in-flight dma_start stalls its
        # issue on an earlier DMA's full completion (measured: 16x1MB
        # pushed the last W arrival from ~54us to ~71us).
        xtiles = {}

        def x_load(t, eng=None):
            xt = ldx.tile([128, D_IN], F32, tag="ldx", name=f"x{t}")
            (eng or nc.sync).dma_start(xt, x_d[ts(t, 128), :])
            xtiles[t] = xt

        x_load(0)
        x_load(1)

        NCH = KNOBS["w_chunks"]
        CHW = (NB // NCH) * D_OUT            # chunk width in f32 columns
        W1 = wres.tile([128, NB * D_OUT], F32, tag="W1", name="W1")
        for ch in range(NCH):
            nc.sync.dma_start(
                W1[:, ds(ch * CHW, CHW)], w_d[:, ds(ch * CHW, CHW)]
            )
        wt = [W1[:, ds(b * D_OUT, D_OUT)] for b in range(NB)]

        cpos = consts.tile([128, 1], F32, tag="cpos")
        nc.vector.memset(cpos, CM)
        czero = consts.tile([128, 1], F32, tag="czero")
        nc.vector.memset(czero, 0.0)
        # dummy activation on a ready constant: triggers the one-time
        # ACT_TABLE_LOAD during DMA warmup instead of on the critical chain
        warm = stats.tile([128, 1], F32, tag="warm")
        nc.scalar.activation(warm, czero, AF.Abs, bias=czero)

        # pass-1 abs-sums: even j on DVE (reduce, non-clobbering), odd j on
        # ACT (Abs -> throwaway bf16 scratch in the idle ys ring, accum_out
        # catches the column sum) so neither engine gates the W stream.
        wsumsD = stats.tile([128, NB // 2], F32, tag="wsumsD")
        wsumsA = stats.tile([128, NB // 2], F32, tag="wsumsA")

        def pass1(j):
            if j % 2 == 0:
                nc.vector.reduce_sum(
                    wsumsD[:, ds(j // 2, 1)], wt[j], axis=AX.X,
                    apply_absolute_value=True,
                )
            else:
                scr = ysp.tile([128, D_OUT], BF16, tag="ys", name=f"p1scr{j}")
                nc.scalar.activation(
                    scr, wt[j], AF.Abs, bias=czero,
                    accum_out=wsumsA[:, ds(j // 2, 1)],
                )

        xscales = {}

        def x_stats(t):
            a = stats.tile([128, 1], F32, tag="xa", name=f"xa{t}")
            nc.vector.reduce_max(a, xtiles[t], axis=AX.X, apply_absolute_value=True)
            nc.vector.tensor_scalar(a, a, 1e-8, None, OP.max)
            r0 = stats.tile([128, 1], F32, tag="xr0", name=f"xr0{t}")
            nc.vector.reciprocal(r0, a)
            ntt = stats.tile([128, 1], F32, tag="xntt", name=f"xntt{t}")
            nc.vector.tensor_mul(ntt, a, r0)
            nc.vector.tensor_scalar(ntt, ntt, -1.0, 2.0, OP.mult, OP.add)
            s = stats.tile([128, 1], F32, tag="xs", name=f"xs{t}")
            nc.vector.tensor_mul(s, r0, ntt)
            nc.vector.tensor_scalar(s, s, QMAX, None, OP.mult)  # 127/a
            xscales[t] = (a, s)

        xqts = {}

        def x_quant(t):
            a, s = xscales[t]
            xt = xtiles.pop(t)
            xq = xqp.tile([128, D_IN], BF16, tag="xq", name=f"xq{t}")
            for h in range(2):
                t1 = t1p.tile([128, HALF], F32, tag="t1", name=f"xt1_{t}_{h}")
                nc.scalar.activation(
                    t1, xt[:, ds(h * HALF, HALF)], AF.Identity, bias=cpos, scale=s
                )
                nc.vector.tensor_scalar(
                    xq[:, ds(h * HALF, HALF)], t1, -CM, None, OP.add
                )
            xqT = xqtp.tile([128, NB, 128], BF16, tag="xqT", name=f"xqT{t}")
            nc.sync.dma_start(xqT, xq, transpose=True)
            xqts[t] = xqT

        souts = {}

        def x_sout(t):
            a, _ = xscales[t]
            so = stats.tile([128, 1], F32, tag="xso", name=f"xso{t}")
            nc.vector.tensor_scalar(so, a, ws127, None, OP.mult)
            souts[t] = so

        # pass-1 sums get high priority so the scheduler never parks them
        # behind the x chains. (Interleaving the x prep INSIDE this block
        # was tried twice -- priority reset AND offset=8 -- and both made
        # the schedule noisier/slower: the scheduler is at a local optimum.)
        with tc.high_priority():
            for j in range(NB):
                pass1(j)
        x_stats(0)
        x_quant(0)
        x_stats(1)
        x_quant(1)

        # ---- w_scale ----
        wsD = stats.tile([128, 1], F32, tag="wsD")
        nc.vector.reduce_sum(wsD, wsumsD, axis=AX.X)
        wsA = stats.tile([128, 1], F32, tag="wsA")
        nc.vector.reduce_sum(wsA, wsumsA, axis=AX.X)
        wsum_p = stats.tile([128, 1], F32, tag="wsp")
        nc.vector.tensor_add(wsum_p, wsD, wsA)
        wsum_all = stats.tile([128, 1], F32, tag="wsa")
        nc.gpsimd.partition_all_reduce(wsum_all, wsum_p, 128, bass_isa.ReduceOp.add)
        wscale = consts.tile([128, 1], F32, tag="wscale")
        nc.vector.tensor_scalar(
            wscale, wsum_all, 1.0 / (D_OUT * D_IN), 1e-6, OP.mult, OP.max
        )
        r0 = stats.tile([128, 1], F32, tag="wr0")
        nc.vector.reciprocal(r0, wscale)
        ntt = stats.tile([128, 1], F32, tag="wntt")
        nc.vector.tensor_mul(ntt, wscale, r0)
        nc.vector.tensor_scalar(ntt, ntt, -1.0, 2.0, OP.mult, OP.add)
        rws = consts.tile([128, 1], F32, tag="rws")
        nc.vector.tensor_mul(rws, r0, ntt)
        ws127 = consts.tile([128, 1], F32, tag="ws127")
        nc.vector.tensor_scalar(ws127, wscale, 1.0 / QMAX, None, OP.mult)
        x_sout(0)
        x_sout(1)

        # x tiles 2,3: issue is gated on ldx slots freeing (~15-20us), so
        # these land on the sync ring BEHIND the W chunks and complete
        # right after the stream -- in time for prep during/after the chase.
        x_load(2)
        x_load(3)

        # ---- W quantize stream + PE chase-ramp ----
        wq = [
            wqp.tile([128, D_OUT], FP8, tag=f"wq{b}", name=f"wq{b}")
            for b in range(NB)
        ]

        def w_quant(b):
            for h in range(2):
                t1 = t1p.tile([128, HALF], F32, tag="t1", name=f"wt1_{b}_{h}")
                nc.scalar.activation(
                    t1, wt[b][:, ds(h * HALF, HALF)], AF.Identity,
                    bias=cpos, scale=rws,
                )
                nc.vector.tensor_scalar(
                    wq[b][:, ds(h * HALF, HALF)], t1, -CM, None, OP.add
                )
            clamp_eng.tensor_scalar(wq[b], wq[b], -1.0, 1.0, OP.max, OP.min)

        # The wq stream is high priority (it gates the PE chase); x tiles
        # 2,3 prep threads through its engine gaps (ACT/DVE have ~0.5us/b
        # of slack vs the chase) so their xqT are ready before the chase
        # ends and the steady state starts without a stall.
        with tc.high_priority():
            for b in range(NB):
                w_quant(b)
        x_stats(2)
        x_quant(2)
        x_sout(2)

        # 8 PSUM cells (tiles 0-1 x col-blocks 0-3) accumulate each b as its
        # wq lands; PE consumes at ~1.7us/b vs ~2.2us/b quantize rate.
        chase = [(t, no) for t in range(2) for no in range(NO)]
        pss = {}
        for c, (t, no) in enumerate(chase):
            pss[c] = psum.tile([128, 512], F32, tag="ps", name=f"cps{c}")
        for b in range(NB):
            for c, (t, no) in enumerate(chase):
                nc.tensor.matmul(
                    pss[c],
                    xqts[t][:, b, :],
                    wq[b][:, ds(no * 512, 512)],
                    start=(b == 0),
                    stop=(b == NB - 1),
                )

        ys = {}

        def y_tile(t):
            if t not in ys:
                ys[t] = ysp.tile([128, D_OUT], BF16, tag="ys", name=f"ys{t}")
            return ys[t]

        # chase drains run on ACT (it can read PSUM): at the chase->steady
        # boundary the DVE still has the wq-stream tail queued, and drains
        # stuck behind it would stall PSUM recycling for the first steady
        # cells.
        for c, (t, no) in enumerate(chase):
            nc.scalar.activation(
                y_tile(t)[:, ds(no * 512, 512)], pss[c], AF.Identity,
                bias=czero, scale=souts[t],
            )
        del pss

        def y_store(t):
            nc.sync.dma_start(y_d[ts(t, 128), :], ys.pop(t))
            del xqts[t]

        y_store(0)
        y_store(1)

        # ---- steady state: x-prep two tiles ahead ----
        def cell(no, t, store_quarter=False):
            ps = psum.tile([128, 512], F32, tag="ps")
            xqT = xqts[t]
            for b in range(NB):
                nc.tensor.matmul(
                    ps,
                    xqT[:, b, :],
                    wq[b][:, ds(no * 512, 512)],
                    start=(b == 0),
                    stop=(b == NB - 1),
                )
            nc.vector.tensor_scalar(
                y_tile(t)[:, ds(no * 512, 512)], ps, souts[t], None, OP.mult
            )
            if store_quarter:
                nc.sync.dma_start(
                    y_d[ts(t, 128), ds(no * 512, 512)],
                    ys[t][:, ds(no * 512, 512)],
                )

        # steady x loads ride the scalar HWDGE ring: the sync ring then
        # carries only transposes + y stores, so a prep transpose is never
        # serialized behind a 1MB in-flight x load (DMA-transpose is
        # ordered against prior DMAs on its ring).
        for t in range(2, NT):
            last = t == NT - 1
            if t + 2 < NT:
                x_load(t + 2, eng=nc.scalar)
            if t + 1 < NT:
                x_stats(t + 1)
                x_quant(t + 1)
                x_sout(t + 1)
            for no in range(NO):
                cell(no, t, store_quarter=last)
            if last:
                ys.pop(t)
                del xqts[t]
            else:
                y_store(t)


def _build():
    key = tuple(sorted((k, str(v)) for k, v in KNOBS.items()))
    if key in _CACHE:
        return _CACHE[key]
    nc = bacc.Bacc(
        "TRN2", target_bir_lowering=False, debug=False, num_devices=N_CORES
    )
    x_d = nc.dram_tensor("x", [TPC, D_IN], F32, kind="ExternalInput").ap()
    # w is fed pre-tiled by the host: w2[p, b*2048 + c] = W[c, b*128 + p]
    w_d = nc.dram_tensor("w", [128, NB * D_OUT], F32, kind="ExternalInput").ap()
    y_d = nc.dram_tensor("y", [TPC, D_OUT], BF16, kind="ExternalOutput").ap()
    with tile.TileContext(nc) as tc:
        _emit(tc, x_d, w_d, y_d)
    nc.compile()
    _CACHE[key] = nc
    return nc


_last_result = None  # BassKernelResults of the most recent run (for profiling)


def kernel(x: np.ndarray, weight: np.ndarray, trace: bool = False) -> np.ndarray:
    global _last_result
    nc = _build()
    xf = np.ascontiguousarray(x.reshape(TOK, D_IN), dtype=np.float32)
    wT2 = np.ascontiguousarray(
        weight.T.astype(np.float32)
        .reshape(NB, 128, D_OUT).transpose(1, 0, 2).reshape(128, NB * D_OUT)
    )
    in_maps = [
        {"x": xf[c * TPC:(c + 1) * TPC], "w": wT2}
        for c in range(N_CORES)
    ]
    res = run_bass_kernel_spmd(nc, in_maps, list(range(N_CORES)), trace=trace)
    _last_result = res
    y = np.concatenate(
        [np.asarray(res.results[c]["y"]) for c in range(N_CORES)], axis=0
    )
    return y.reshape(B, S, D_OUT).astype(np.float32)
